# revision 1
# baseline (speedup 1.0000x reference)
"""MultiHeadAttention Trainium2 kernel.

Sharding: 8 cores = 4 batches x 2 head-halves. Core c handles batch c//2,
heads (c%2)*8 .. +8. Each core computes a partial output projection
(its 8 heads' contribution, 512 of the 1024 contraction dims of Wo);
the host sums the two partials per batch and adds the bias.

Per-core dataflow (S=2048 seq, D=1024 model, 8 local heads of 64):
  phase 1: PE-transpose each input X (query/key/value) tile-by-tile to get
           X^T, then project:
             q^T (zero-padded per head to 128 partitions, for head-pair
                  packed score matmuls), k^T (head-pair layout), and
             v_aug (v chunks with a ones column appended -> softmax
                  denominator comes free out of the ctx matmul).
  phase 2: per (q-tile of 512, head): scores^T = k^T_pair.T @ q_pad
           (PSUM), exp on ACT straight PSUM->SBUF (scale=1/sqrt(D) folded
           into the activation), ctx^T[65,512] = [v|1]^T @ E accumulated
           over 16 key chunks; row 64 is the softmax denominator.
           Normalize: 1/D = exp(-ln(D)) on ACT (ln+exp share one table
           set), broadcast across partitions via a ones[1,128] PE matmul,
           then one DVE multiply per head. Output projection per q-tile.
All matmuls run as float32r (~1.4e-4 rel err, 4x faster than fp32 mode).
"""

import os
import sys

# The bass kernel needs the TRN (axon) jax backend; if the caller pinned
# JAX_PLATFORMS=cpu for its reference computation, undo that before jax
# gets imported (no-op if jax is already initialized).
if os.environ.get("JAX_PLATFORMS") == "cpu":
    del os.environ["JAX_PLATFORMS"]

sys.path.insert(0, "/opt/trn_rl_repo")

import numpy as np
import ml_dtypes

import concourse.bass as bass
import concourse.mybir as mybir
import concourse.tile as tile
from concourse import bacc
from concourse.bass_utils import run_bass_kernel_spmd
from concourse.masks import make_identity
import concourse.hw_specs as hw_specs

_orig_get_act_tables = hw_specs.get_activation_tables


def _only_ln_exp_set(module_arch):
    # Keep all set names in original order (set_id = dict index), but leave
    # only natural_log_exp_and_others non-empty so every activation resolves
    # to that one set -> a single ACT_TABLE_LOAD for exp+ln+copy.
    t = _orig_get_act_tables(module_arch)
    name = "natural_log_exp_and_others"
    assert name in t, list(t)
    return {k: (v if k == name else set()) for k, v in t.items()}


hw_specs.get_activation_tables = _only_ln_exp_set
bacc.get_activation_tables = _only_ln_exp_set

f32 = mybir.dt.float32
f32r = mybir.dt.float32r
P = 128
S = 2048
D = 1024
HLOC = 8          # heads per core
HK = 64           # head size
DKL = HLOC * HK   # local projected dims = 512
QT = 512          # q-tile size
NQT = S // QT     # 4
NKC = S // P      # 16 key chunks
EXP = mybir.ActivationFunctionType.Exp
LN = mybir.ActivationFunctionType.Ln
SCALE = 1.0 / np.sqrt(np.float32(D))  # = 1/32, exact


def _round_f32r(a):
    hi = a.astype(ml_dtypes.bfloat16).astype(np.float32)
    lo = (a - hi).astype(ml_dtypes.bfloat16).astype(np.float32)
    return hi + lo


def build_nc():
    nc = bacc.Bacc("TRN2", target_bir_lowering=False)
    xq = nc.dram_tensor("xq", [S, D], f32, kind="ExternalInput")
    xk = nc.dram_tensor("xk", [S, D], f32, kind="ExternalInput")
    xv = nc.dram_tensor("xv", [S, D], f32, kind="ExternalInput")
    wq = nc.dram_tensor("wq", [D, DKL], f32r, kind="ExternalInput")
    wk = nc.dram_tensor("wk", [D, DKL], f32r, kind="ExternalInput")
    wv = nc.dram_tensor("wv", [D, DKL], f32r, kind="ExternalInput")
    wot = nc.dram_tensor("wot", [DKL, D], f32r, kind="ExternalInput")
    out = nc.dram_tensor("out", [S, D], f32, kind="ExternalOutput")

    ds, ts = bass.ds, bass.ts

    with tile.TileContext(nc) as tc:
        with tc.tile_pool(name="persist", bufs=1) as pp:
            # head h's q^T, zero-padded to the 128-partition pair layout
            qpad = pp.tile([P, HLOC, S], f32r, name="qpad")
            # k^T in head-pair layout: [:, p, :] rows 0-63 = head 2p, 64-127 = 2p+1
            kT = pp.tile([P, HLOC // 2, S], f32r, name="kT")
            # v chunks + ones column: [k-part, chunk, head, 64 v | 1]
            vaug = pp.tile([P, NKC, HLOC, HK + 1], f32r, name="vaug")
            idn = pp.tile([P, P], f32, name="idn")
            ones1 = pp.tile([1, P], f32r, name="ones1")
            nc.vector.memset(ones1[:].bitcast(f32), 1.0)

            make_identity(nc, idn[:])
            nc.vector.memset(qpad[:].bitcast(f32), 0.0)
            nc.vector.memset(vaug[:].bitcast(f32), 1.0)

            # ---------------- phase 1: transpose + projections ----------------
            with tc.tile_pool(name="p1sb", bufs=1) as p1, tc.tile_pool(
                name="tp_ps", bufs=3, space="PSUM"
            ) as tp_ps, tc.tile_pool(name="pr_ps", bufs=3, space="PSUM") as pr_ps:
                for t, (xdram, wdram) in enumerate(((xq, wq), (xk, wk), (xv, wv))):
                    w_sb = p1.tile([P, D // P, DKL], f32r, name=f"w{t}", tag="w", bufs=2)
                    nc.sync.dma_start(
                        w_sb[:], wdram.rearrange("(dc p) n -> p dc n", p=P)
                    )
                    for st in range(S // QT):
                        # stage 4 row-blocks of X
                        stg = []
                        for sb in range(4):
                            xst = p1.tile([P, D], f32, name="xst", tag="xst", bufs=5)
                            nc.sync.dma_start(
                                xst[:], xdram[ds(st * QT + sb * P, P), :]
                            )
                            stg.append(xst)
                        # transpose to X^T tile [128 d, dchunk, 512 s]
                        xt = p1.tile([P, D // P, QT], f32r, name="xt", tag="xt")
                        for dc in range(D // P):
                            ptp = tp_ps.tile([P, QT], f32, name="ptp")
                            for sb in range(4):
                                nc.tensor.transpose(
                                    ptp[:, ds(sb * P, P)],
                                    stg[sb][:, ds(dc * P, P)],
                                    idn[:],
                                )
                            nc.scalar.copy(xt[:, dc, :], ptp[:])
                        if t < 2:
                            # q^T / k^T orientation: psum [128 dk, 512 s]
                            for dkb in range(DKL // P):
                                ppr = pr_ps.tile([P, QT], f32, name="ppr")
                                for dc in range(D // P):
                                    nc.tensor.matmul(
                                        ppr[:],
                                        w_sb[:, dc, ds(dkb * P, P)],
                                        xt[:, dc, :],
                                        start=(dc == 0),
                                        stop=(dc == D // P - 1),
                                    )
                                if t == 0:
                                    nc.vector.tensor_copy(
                                        qpad[0:64, 2 * dkb, ds(st * QT, QT)],
                                        ppr[0:64, :],
                                    )
                                    nc.vector.tensor_copy(
                                        qpad[64:128, 2 * dkb + 1, ds(st * QT, QT)],
                                        ppr[64:128, :],
                                    )
                                else:
                                    nc.vector.tensor_copy(
                                        kT[:, dkb, ds(st * QT, QT)], ppr[:]
                                    )
                        else:
                            # v orientation: psum [128 s, 512 dk]
                            for sb in range(4):
                                ppr = pr_ps.tile([P, DKL], f32, name="ppr")
                                for dc in range(D // P):
                                    nc.tensor.matmul(
                                        ppr[:],
                                        xt[:, dc, ds(sb * P, P)],
                                        w_sb[:, dc, :],
                                        start=(dc == 0),
                                        stop=(dc == D // P - 1),
                                    )
                                ci = st * 4 + sb
                                nc.vector.tensor_copy(
                                    vaug[:, ci, :, 0:HK],
                                    ppr[:].rearrange("p (h k) -> p h k", k=HK),
                                )

            # ---------------- phase 2: attention + output projection ----------
            with tc.tile_pool(name="p2sb", bufs=1) as p2, tc.tile_pool(
                name="et_pool", bufs=3
            ) as etp, tc.tile_pool(name="bc_pool", bufs=2) as bcp, tc.tile_pool(
                name="cu_pool", bufs=1
            ) as cup, tc.tile_pool(name="ct_pool", bufs=2) as ctp, tc.tile_pool(
                name="ob_pool", bufs=2
            ) as obp, tc.tile_pool(
                name="sc_ps", bufs=2, space="PSUM"
            ) as sc_ps, tc.tile_pool(
                name="ct_ps", bufs=1, space="PSUM"
            ) as ct_ps, tc.tile_pool(
                name="op_ps", bufs=1, space="PSUM"
            ) as op_ps, tc.tile_pool(
                name="bc_ps", bufs=2, space="PSUM"
            ) as bc_ps:
                wot_sb = p2.tile([P, DKL // P, D], f32r, name="wot_sb")
                nc.sync.dma_start(
                    wot_sb[:], wot.rearrange("(c p) n -> p c n", p=P)
                )
                for qt in range(NQT):
                    ctxU = cup.tile([P, DKL // P, QT], f32, name="ctxU")
                    ctxT = ctp.tile([P, DKL // P, QT], f32r, name="ctxT")
                    for h in range(HLOC):
                        p = h // 2
                        pct = ct_ps.tile([HK + 1, QT], f32, name="pct")
                        for g in range(NKC // 2):
                            psc = sc_ps.tile([P, 2, QT], f32, name="psc")
                            for j in range(2):
                                ch = 2 * g + j
                                nc.tensor.matmul(
                                    psc[:, j, :],
                                    kT[:, p, ds(ch * P, P)],
                                    qpad[:, h, ds(qt * QT, QT)],
                                    start=True,
                                    stop=True,
                                )
                            ete = etp.tile([P, 2, QT], f32r, name="ete")
                            nc.scalar.activation(ete[:], psc[:], EXP, scale=SCALE)
                            for j in range(2):
                                ch = 2 * g + j
                                nc.tensor.matmul(
                                    pct[:],
                                    vaug[:, ch, h, :],
                                    ete[:, j, :],
                                    start=(ch == 0),
                                    stop=(ch == NKC - 1),
                                )
                        sl = slice(64 * (h % 2), 64 * (h % 2) + 64)
                        nc.vector.tensor_copy(ctxU[sl, h // 2, :], pct[0:64, :])
                        dln = bcp.tile([1, QT], f32, name="dln", tag="dln", bufs=3)
                        nc.scalar.activation(dln[:], pct[64:65, :], LN)
                        drec = bcp.tile([1, QT], f32r, name="drec", tag="drec", bufs=3)
                        nc.scalar.activation(drec[:], dln[:], EXP, scale=-1.0)
                        bch = bc_ps.tile([P, QT], f32, name="bch")
                        nc.tensor.matmul(bch[:], ones1[:], drec[:], start=True, stop=True)
                        nc.vector.tensor_mul(
                            out=ctxT[sl, h // 2, :],
                            in0=ctxU[sl, h // 2, :],
                            in1=bch[sl, :],
                        )
                    for sb in range(QT // P):
                        ob = obp.tile([P, 2, QT], f32, name="ob")
                        for nh in range(2):
                            pop = op_ps.tile([P, QT], f32, name="pop")
                            for cc in range(DKL // P):
                                nc.tensor.matmul(
                                    pop[:],
                                    ctxT[:, cc, ds(sb * P, P)],
                                    wot_sb[:, cc, ds(nh * QT, QT)],
                                    start=(cc == 0),
                                    stop=(cc == DKL // P - 1),
                                )
                            nc.vector.tensor_copy(ob[:, nh, :], pop[:])
                        nc.sync.dma_start(
                            out[ds(qt * QT + sb * P, P), :],
                            ob[:].rearrange("p a b -> p (a b)"),
                        )
    nc.compile()
    return nc


_NC_CACHE = []


def _get_nc():
    if not _NC_CACHE:
        _NC_CACHE.append(build_nc())
    return _NC_CACHE[0]


def make_in_maps(query, key_in, value, Wq, Wk, Wv, Wo, bo):
    query = np.asarray(query, dtype=np.float32)
    key_in = np.asarray(key_in, dtype=np.float32)
    value = np.asarray(value, dtype=np.float32)
    Wq = np.asarray(Wq, dtype=np.float32)
    Wk = np.asarray(Wk, dtype=np.float32)
    Wv = np.asarray(Wv, dtype=np.float32)
    Wo = np.asarray(Wo, dtype=np.float32)
    in_maps = []
    for c in range(8):
        b = c // 2
        h0 = (c % 2) * HLOC
        wq_c = _round_f32r(
            np.ascontiguousarray(
                Wq[h0 : h0 + HLOC].transpose(1, 0, 2).reshape(D, DKL)
            )
        )
        wk_c = _round_f32r(
            np.ascontiguousarray(
                Wk[h0 : h0 + HLOC].transpose(1, 0, 2).reshape(D, DKL)
            )
        )
        wv_c = _round_f32r(
            np.ascontiguousarray(
                Wv[h0 : h0 + HLOC].transpose(1, 0, 2).reshape(D, DKL)
            )
        )
        wot_c = _round_f32r(
            np.ascontiguousarray(Wo[:, h0 * HK : h0 * HK + DKL].T)
        )
        in_maps.append(
            {
                "xq": np.ascontiguousarray(query[b]),
                "xk": np.ascontiguousarray(key_in[b]),
                "xv": np.ascontiguousarray(value[b]),
                "wq": wq_c,
                "wk": wk_c,
                "wv": wv_c,
                "wot": wot_c,
            }
        )
    return in_maps


def kernel(query, key_in, value, Wq, Wk, Wv, Wo, bo):
    nc = _get_nc()
    in_maps = make_in_maps(query, key_in, value, Wq, Wk, Wv, Wo, bo)
    res = run_bass_kernel_spmd(nc, in_maps, list(range(8)))
    bo = np.asarray(bo, dtype=np.float32)
    B = np.asarray(query).shape[0]
    out = np.empty((B, S, D), dtype=np.float32)
    for b in range(B):
        out[b] = res.results[2 * b]["out"] + res.results[2 * b + 1]["out"] + bo
    return out


if __name__ == "__main__":
    rng = np.random.default_rng(0)
    q = rng.standard_normal((4, S, D), dtype=np.float32)
    k = rng.standard_normal((4, S, D), dtype=np.float32)
    v = rng.standard_normal((4, S, D), dtype=np.float32)
    sd = 1.0 / np.sqrt(D)
    Wq = rng.standard_normal((16, D, HK), dtype=np.float32) * sd
    Wk = rng.standard_normal((16, D, HK), dtype=np.float32) * sd
    Wv = rng.standard_normal((16, D, HK), dtype=np.float32) * sd
    Wo = rng.standard_normal((D, D), dtype=np.float32) * sd
    bo = rng.standard_normal((D,), dtype=np.float32) * 0.01
    o = kernel(q, k, v, Wq, Wk, Wv, Wo, bo)
    print("out", o.shape, o.dtype, np.abs(o).max())



# revision 2
# speedup vs baseline: 2.2790x; 2.2790x over previous
"""MultiHeadAttention Trainium2 kernel.

Sharding: 8 cores = 4 batches x 2 query-halves. Core c handles batch c//2,
query rows (c%2)*1024 .. +1024, ALL 16 heads. Each core computes its full
output rows (complete Wo contraction + bias on device) -> no host-side
reduction, just concatenation.

Weights are identical on every core, so they are baked into the program as
Const (inline_tensor) data in fp16 -- loaded once at model load, never
shipped per call. Per-call traffic is only activations: xq-half (2MB) +
xk/xv full (4MB each) in fp16, and the 2MB fp16 output tile.

Per-core dataflow (S_q=1024 local queries, S_k=2048 keys, D=1024):
  phase 1: PE-transpose each input X tile-by-tile to X^T, then project:
           q^T zero-padded per head into a 128-partition head-pair layout,
           k^T in head-pair layout, v_aug (v chunks + ones column so the
           softmax denominator falls out of the ctx matmul).
  phase 2: per (q-tile of 512, head): scores^T = k^T_pair.T @ q_pad (PSUM),
           exp on ACT straight PSUM->SBUF (1/sqrt(D) folded into the
           activation scale), ctx^T[65,512] = [v|1]^T @ E accumulated over
           16 key chunks; row 64 is the softmax denominator. Normalize via
           ln+exp on ACT, broadcast with a ones[1,128] PE matmul, one DVE
           multiply per head. Output projection per q-tile starts each PSUM
           accumulation with a rank-1 bias matmul (ones^T @ bo) so the bias
           is added for free.
All matmuls run in fp16 (x and weights are shipped/baked fp16; scores/ctx
operands stored fp16; PSUM accumulation fp32) -> rel err ~1e-3 vs fp32.
"""

import os
import sys

# The bass kernel needs the TRN (axon) jax backend; if the caller pinned
# JAX_PLATFORMS=cpu for its reference computation, undo that before jax
# gets imported (no-op if jax is already initialized).
if os.environ.get("JAX_PLATFORMS") == "cpu":
    del os.environ["JAX_PLATFORMS"]

sys.path.insert(0, "/opt/trn_rl_repo")

import numpy as np

import concourse.bass as bass
import concourse.mybir as mybir
import concourse.tile as tile
from concourse import bacc
from concourse.bass_utils import run_bass_kernel_spmd
from concourse.masks import make_identity
import concourse.hw_specs as hw_specs

_orig_get_act_tables = hw_specs.get_activation_tables


def _only_ln_exp_set(module_arch):
    # Keep all set names in original order (set_id = dict index), but leave
    # only natural_log_exp_and_others non-empty so every activation resolves
    # to that one set -> a single ACT_TABLE_LOAD for exp+ln+copy.
    t = _orig_get_act_tables(module_arch)
    name = "natural_log_exp_and_others"
    assert name in t, list(t)
    return {k: (v if k == name else set()) for k, v in t.items()}


hw_specs.get_activation_tables = _only_ln_exp_set
bacc.get_activation_tables = _only_ln_exp_set

f16 = mybir.dt.float16
f32 = mybir.dt.float32
P = 128
S = 2048          # full sequence (keys)
SQ = 1024         # local query rows per core
D = 1024
NH = 16           # heads (all on every core)
HK = 64           # head size
QT = 512          # q-tile size
NQT = SQ // QT    # 2
NKC = S // P      # 16 key chunks
EXP = mybir.ActivationFunctionType.Exp
LN = mybir.ActivationFunctionType.Ln
SCALE = 1.0 / np.sqrt(np.float32(D))  # = 1/32, exact


def build_nc(WQ, WK, WV, WOT, BO):
    """WQ/WK/WV: [D, NH*HK] fp16 head-stacked; WOT: [D, D] fp16 = Wo.T;
    BO: [1, D] fp16."""
    nc = bacc.Bacc("TRN2", target_bir_lowering=False)
    xq = nc.dram_tensor("xq", [SQ, D], f16, kind="ExternalInput")
    xk = nc.dram_tensor("xk", [S, D], f16, kind="ExternalInput")
    xv = nc.dram_tensor("xv", [S, D], f16, kind="ExternalInput")
    wq = nc.inline_tensor(WQ, name="wq")
    wk = nc.inline_tensor(WK, name="wk")
    wv = nc.inline_tensor(WV, name="wv")
    wot = nc.inline_tensor(WOT, name="wot")
    bo = nc.inline_tensor(BO, name="bo")
    out = nc.dram_tensor("out", [SQ, D], f16, kind="ExternalOutput")

    ds = bass.ds

    with tile.TileContext(nc) as tc:
        with tc.tile_pool(name="persist", bufs=1) as pp:
            # head h's q^T, zero-padded to the 128-partition pair layout
            qpad = pp.tile([P, NH, SQ], f16, name="qpad")
            # k^T in head-pair layout: [:, p, :] rows 0-63 = head 2p, 64-127 = 2p+1
            kT = pp.tile([P, D // P, S], f16, name="kT")
            # v chunks + ones column: [k-part, chunk, head, 64 v | 1]
            vaug = pp.tile([P, NKC, NH, HK + 1], f16, name="vaug")
            idn = pp.tile([P, P], f16, name="idn")
            ones1 = pp.tile([1, P], f16, name="ones1")
            nc.vector.memset(ones1[:], 1.0)

            make_identity(nc, idn[:])
            nc.vector.memset(qpad[:], 0.0)
            nc.vector.memset(vaug[:], 1.0)

            # ---------------- phase 1: transpose + projections ----------------
            with tc.tile_pool(name="p1sb", bufs=1) as p1, tc.tile_pool(
                name="tp_ps", bufs=3, space="PSUM"
            ) as tp_ps, tc.tile_pool(name="pr_ps", bufs=3, space="PSUM") as pr_ps:
                for t, (xdram, wdram, s_rows) in enumerate(
                    ((xq, wq, SQ), (xk, wk, S), (xv, wv, S))
                ):
                    w_sb = p1.tile([P, D // P, D], f16, name=f"w{t}", tag="w", bufs=2)
                    nc.sync.dma_start(
                        w_sb[:], wdram.rearrange("(dc p) n -> p dc n", p=P)
                    )
                    for st in range(s_rows // QT):
                        # stage 4 row-blocks of X
                        stg = []
                        for sb in range(4):
                            xst = p1.tile([P, D], f16, name="xst", tag="xst", bufs=5)
                            nc.sync.dma_start(
                                xst[:], xdram[ds(st * QT + sb * P, P), :]
                            )
                            stg.append(xst)
                        # transpose to X^T tile [128 d, dchunk, 512 s]
                        xt = p1.tile([P, D // P, QT], f16, name="xt", tag="xt", bufs=2)
                        for dc in range(D // P):
                            ptp = tp_ps.tile([P, QT], f16, name="ptp")
                            for sb in range(4):
                                nc.tensor.transpose(
                                    ptp[:, ds(sb * P, P)],
                                    stg[sb][:, ds(dc * P, P)],
                                    idn[:],
                                )
                            nc.scalar.copy(xt[:, dc, :], ptp[:])
                        if t < 2:
                            # q^T / k^T orientation: psum [128 dk, 512 s]
                            for dkb in range(D // P):
                                ppr = pr_ps.tile([P, QT], f32, name="ppr")
                                for dc in range(D // P):
                                    nc.tensor.matmul(
                                        ppr[:],
                                        w_sb[:, dc, ds(dkb * P, P)],
                                        xt[:, dc, :],
                                        start=(dc == 0),
                                        stop=(dc == D // P - 1),
                                    )
                                if t == 0:
                                    nc.vector.tensor_copy(
                                        qpad[0:64, 2 * dkb, ds(st * QT, QT)],
                                        ppr[0:64, :],
                                    )
                                    nc.vector.tensor_copy(
                                        qpad[64:128, 2 * dkb + 1, ds(st * QT, QT)],
                                        ppr[64:128, :],
                                    )
                                else:
                                    nc.vector.tensor_copy(
                                        kT[:, dkb, ds(st * QT, QT)], ppr[:]
                                    )
                        else:
                            # v orientation: psum [128 s, 512 dk] per half
                            for sb in range(4):
                                ci = st * 4 + sb
                                for vh in range(2):
                                    ppr = pr_ps.tile([P, QT], f32, name="ppr")
                                    for dc in range(D // P):
                                        nc.tensor.matmul(
                                            ppr[:],
                                            xt[:, dc, ds(sb * P, P)],
                                            w_sb[:, dc, ds(vh * QT, QT)],
                                            start=(dc == 0),
                                            stop=(dc == D // P - 1),
                                        )
                                    nc.vector.tensor_copy(
                                        vaug[:, ci, ds(vh * 8, 8), 0:HK],
                                        ppr[:].rearrange("p (h k) -> p h k", k=HK),
                                    )

            # ---------------- phase 2: attention + output projection ----------
            with tc.tile_pool(name="p2sb", bufs=1) as p2, tc.tile_pool(
                name="et_pool", bufs=3
            ) as etp, tc.tile_pool(name="bc_pool", bufs=2) as bcp, tc.tile_pool(
                name="cu_pool", bufs=1
            ) as cup, tc.tile_pool(name="ct_pool", bufs=2) as ctp, tc.tile_pool(
                name="ob_pool", bufs=2
            ) as obp, tc.tile_pool(
                name="sc_ps", bufs=2, space="PSUM"
            ) as sc_ps, tc.tile_pool(
                name="ct_ps", bufs=1, space="PSUM"
            ) as ct_ps, tc.tile_pool(
                name="op_ps", bufs=1, space="PSUM"
            ) as op_ps, tc.tile_pool(
                name="bc_ps", bufs=2, space="PSUM"
            ) as bc_ps:
                wot_sb = p2.tile([P, D // P, D], f16, name="wot_sb")
                nc.sync.dma_start(
                    wot_sb[:], wot.rearrange("(c p) n -> p c n", p=P)
                )
                bo_sb = p2.tile([1, D], f16, name="bo_sb")
                nc.sync.dma_start(bo_sb[:], bo[:, :])
                for qt in range(NQT):
                    ctxU = cup.tile([P, D // P, QT], f32, name="ctxU")
                    ctxT = ctp.tile([P, D // P, QT], f16, name="ctxT")
                    for h in range(NH):
                        p = h // 2
                        pct = ct_ps.tile([HK + 1, QT], f32, name="pct")
                        for g in range(NKC // 2):
                            psc = sc_ps.tile([P, 2, QT], f32, name="psc")
                            for j in range(2):
                                ch = 2 * g + j
                                nc.tensor.matmul(
                                    psc[:, j, :],
                                    kT[:, p, ds(ch * P, P)],
                                    qpad[:, h, ds(qt * QT, QT)],
                                    start=True,
                                    stop=True,
                                )
                            ete = etp.tile([P, 2, QT], f16, name="ete")
                            nc.scalar.activation(ete[:], psc[:], EXP, scale=SCALE)
                            for j in range(2):
                                ch = 2 * g + j
                                nc.tensor.matmul(
                                    pct[:],
                                    vaug[:, ch, h, :],
                                    ete[:, j, :],
                                    start=(ch == 0),
                                    stop=(ch == NKC - 1),
                                )
                        sl = slice(64 * (h % 2), 64 * (h % 2) + 64)
                        nc.vector.tensor_copy(ctxU[sl, h // 2, :], pct[0:64, :])
                        dln = bcp.tile([1, QT], f32, name="dln", tag="dln", bufs=3)
                        nc.scalar.activation(dln[:], pct[64:65, :], LN)
                        drec = bcp.tile([1, QT], f16, name="drec", tag="drec", bufs=3)
                        nc.scalar.activation(drec[:], dln[:], EXP, scale=-1.0)
                        bch = bc_ps.tile([P, QT], f32, name="bch")
                        nc.tensor.matmul(bch[:], ones1[:], drec[:], start=True, stop=True)
                        nc.vector.tensor_mul(
                            out=ctxT[sl, h // 2, :],
                            in0=ctxU[sl, h // 2, :],
                            in1=bch[sl, :],
                        )
                    for sb in range(QT // P):
                        ob = obp.tile([P, 2, QT], f16, name="ob")
                        for nh in range(2):
                            pop = op_ps.tile([P, QT], f32, name="pop")
                            # rank-1 bias matmul seeds the accumulation
                            nc.tensor.matmul(
                                pop[:],
                                ones1[:],
                                bo_sb[0:1, ds(nh * QT, QT)],
                                start=True,
                                stop=False,
                            )
                            for cc in range(D // P):
                                nc.tensor.matmul(
                                    pop[:],
                                    ctxT[:, cc, ds(sb * P, P)],
                                    wot_sb[:, cc, ds(nh * QT, QT)],
                                    start=False,
                                    stop=(cc == D // P - 1),
                                )
                            nc.vector.tensor_copy(ob[:, nh, :], pop[:])
                        nc.sync.dma_start(
                            out[ds(qt * QT + sb * P, P), :],
                            ob[:].rearrange("p a b -> p (a b)"),
                        )
    nc.compile()
    return nc


_NC_CACHE = {}


def _weights_key(Wq, Wk, Wv, Wo, bo):
    import hashlib

    h = hashlib.sha1()
    for a in (Wq, Wk, Wv, Wo, bo):
        h.update(np.ascontiguousarray(a).tobytes())
    return h.hexdigest()


def _get_nc(Wq, Wk, Wv, Wo, bo):
    key = _weights_key(Wq, Wk, Wv, Wo, bo)
    if key not in _NC_CACHE:
        WQ = np.ascontiguousarray(
            np.asarray(Wq, np.float32).transpose(1, 0, 2).reshape(D, NH * HK)
        ).astype(np.float16)
        WK = np.ascontiguousarray(
            np.asarray(Wk, np.float32).transpose(1, 0, 2).reshape(D, NH * HK)
        ).astype(np.float16)
        WV = np.ascontiguousarray(
            np.asarray(Wv, np.float32).transpose(1, 0, 2).reshape(D, NH * HK)
        ).astype(np.float16)
        WOT = np.ascontiguousarray(np.asarray(Wo, np.float32).T).astype(np.float16)
        BO = np.asarray(bo, np.float32).reshape(1, D).astype(np.float16)
        _NC_CACHE.clear()  # one compiled program at a time
        _NC_CACHE[key] = build_nc(WQ, WK, WV, WOT, BO)
    return _NC_CACHE[key]


def make_in_maps(query, key_in, value, Wq, Wk, Wv, Wo, bo):
    query = np.asarray(query, dtype=np.float16)
    key_in = np.asarray(key_in, dtype=np.float16)
    value = np.asarray(value, dtype=np.float16)
    in_maps = []
    for c in range(8):
        b = c // 2
        half = c % 2
        in_maps.append(
            {
                "xq": np.ascontiguousarray(query[b, half * SQ : (half + 1) * SQ]),
                "xk": np.ascontiguousarray(key_in[b]),
                "xv": np.ascontiguousarray(value[b]),
            }
        )
    return in_maps


def kernel(query, key_in, value, Wq, Wk, Wv, Wo, bo):
    nc = _get_nc(Wq, Wk, Wv, Wo, bo)
    in_maps = make_in_maps(query, key_in, value, Wq, Wk, Wv, Wo, bo)
    res = run_bass_kernel_spmd(nc, in_maps, list(range(8)))
    B = np.asarray(query).shape[0]
    out = np.empty((B, S, D), dtype=np.float32)
    for c in range(8):
        b = c // 2
        half = c % 2
        out[b, half * SQ : (half + 1) * SQ] = res.results[c]["out"]
    return out


if __name__ == "__main__":
    rng = np.random.default_rng(0)
    q = rng.standard_normal((4, S, D), dtype=np.float32)
    k = rng.standard_normal((4, S, D), dtype=np.float32)
    v = rng.standard_normal((4, S, D), dtype=np.float32)
    sd = 1.0 / np.sqrt(D)
    Wq = rng.standard_normal((16, D, HK), dtype=np.float32) * sd
    Wk = rng.standard_normal((16, D, HK), dtype=np.float32) * sd
    Wv = rng.standard_normal((16, D, HK), dtype=np.float32) * sd
    Wo = rng.standard_normal((D, D), dtype=np.float32) * sd
    bo = rng.standard_normal((D,), dtype=np.float32) * 0.01
    o = kernel(q, k, v, Wq, Wk, Wv, Wo, bo)
    print("out", o.shape, o.dtype, np.abs(o).max())


# revision 7
# speedup vs baseline: 3.2747x; 1.4369x over previous
"""MultiHeadAttention Trainium2 kernel.

Sharding: 8 cores = 4 batches x 2 sequence-halves. Core c handles batch
c//2, sequence rows (c%2)*1024 .. +1024, ALL 16 heads. Each core receives
only ITS half of xq/xk/xv; it projects its key/value half and AllGathers
the projected K^T / V tiles with its pair core over NeuronLink, so every
input byte is shipped from the host exactly once. Each core computes its
full output rows (complete Wo contraction + bias on device) -> no host-side
reduction, just concatenation.

Weights are identical on every core, so they are baked into the program as
Const (inline_tensor) data in fp16 -- loaded once at model load, never
shipped per call. Per-call traffic is only activations: xq/xk/xv halves
(2MB each) in fp16, and the 2MB fp16 output tile.

Per-core dataflow (S_q=1024 local queries, S_k=2048 keys, D=1024):
  phase 1: PE-transpose each input X tile-by-tile to X^T, then project:
           q^T zero-padded per head into a 128-partition head-pair layout,
           k^T in head-pair layout, v_aug (v chunks + ones column so the
           softmax denominator falls out of the ctx matmul).
  phase 2: per (q-tile of 512, head): scores^T = k^T_pair.T @ q_pad (PSUM),
           exp on ACT straight PSUM->SBUF (1/sqrt(D) folded into the
           activation scale), ctx^T[65,512] = [v|1]^T @ E accumulated over
           16 key chunks; row 64 is the softmax denominator. Normalize via
           ln+exp on ACT, broadcast with a ones[1,128] PE matmul, one DVE
           multiply per head. Output projection per q-tile starts each PSUM
           accumulation with a rank-1 bias matmul (ones^T @ bo) so the bias
           is added for free.
All matmuls run in fp16 (x and weights are shipped/baked fp16; scores/ctx
operands stored fp16; PSUM accumulation fp32) -> rel err ~1e-3 vs fp32.
"""

import os
import sys

# The bass kernel needs the TRN (axon) jax backend; if the caller pinned
# JAX_PLATFORMS=cpu for its reference computation, undo that before jax
# gets imported (no-op if jax is already initialized).
if os.environ.get("JAX_PLATFORMS") == "cpu":
    del os.environ["JAX_PLATFORMS"]

sys.path.insert(0, "/opt/trn_rl_repo")

import numpy as np

import concourse.bass as bass
import concourse.mybir as mybir
import concourse.tile as tile
from concourse import bacc
from concourse.bass_utils import run_bass_kernel_spmd
from concourse.masks import make_identity
import concourse.hw_specs as hw_specs

_orig_get_act_tables = hw_specs.get_activation_tables


def _only_ln_exp_set(module_arch):
    # Keep all set names in original order (set_id = dict index), but leave
    # only natural_log_exp_and_others non-empty so every activation resolves
    # to that one set -> a single ACT_TABLE_LOAD for exp+ln+copy.
    t = _orig_get_act_tables(module_arch)
    name = "natural_log_exp_and_others"
    assert name in t, list(t)
    return {k: (v if k == name else set()) for k, v in t.items()}


hw_specs.get_activation_tables = _only_ln_exp_set
bacc.get_activation_tables = _only_ln_exp_set

f16 = mybir.dt.float16
f32 = mybir.dt.float32
P = 128
S = 2048          # full sequence (keys)
SQ = 1024         # local query rows per core
D = 1024
NH = 16           # heads (all on every core)
HK = 64           # head size
QT = 512          # q-tile size
NQT = SQ // QT    # 2
NKC = S // P      # 16 key chunks
EXP = mybir.ActivationFunctionType.Exp
LN = mybir.ActivationFunctionType.Ln
SCALE = 1.0 / np.sqrt(np.float32(D))  # = 1/32, exact


def build_nc(WQ, WK, WV, WOT, BO):
    """WQ/WK/WV: [D, NH*HK] fp16 head-stacked; WOT: [D, D] fp16 = Wo.T;
    BO: [1, D] fp16."""
    nc = bacc.Bacc("TRN2", target_bir_lowering=False, num_devices=8)
    xq = nc.dram_tensor("xq", [SQ, D], f16, kind="ExternalInput")
    xk = nc.dram_tensor("xk", [SQ, D], f16, kind="ExternalInput")
    xv = nc.dram_tensor("xv", [SQ, D], f16, kind="ExternalInput")
    wq = nc.inline_tensor(WQ, name="wq")
    wk = nc.inline_tensor(WK, name="wk")
    wv = nc.inline_tensor(WV, name="wv")
    wot = nc.inline_tensor(WOT, name="wot")
    bo = nc.inline_tensor(BO, name="bo")
    out = nc.dram_tensor("out", [SQ, D], f16, kind="ExternalOutput")

    ds = bass.ds

    with tile.TileContext(nc) as tc:
        with tc.tile_pool(name="persist", bufs=1) as pp:
            # head h's q^T, zero-padded to the 128-partition pair layout
            qpad = pp.tile([P, NH, SQ], f16, name="qpad")
            # k^T in head-pair layout: [:, p, :] rows 0-63 = head 2p, 64-127 = 2p+1
            kT = pp.tile([P, D // P, S], f16, name="kT")
            # v chunks + ones column: [k-part, chunk, head, 64 v | 1]
            vaug = pp.tile([P, NKC, NH, HK + 1], f16, name="vaug")
            idn = pp.tile([P, P], f16, name="idn")
            ones1 = pp.tile([1, P], f16, name="ones1")
            nc.vector.memset(ones1[:], 1.0)

            make_identity(nc, idn[:])
            nc.vector.memset(qpad[:], 0.0)
            nc.vector.memset(vaug[:], 1.0)

            # ---------------- phase 1: transpose + projections ----------------
            # k/v first so their pair AllGather overlaps the q projection.
            with tc.tile_pool(name="p1sb", bufs=1) as p1, tc.tile_pool(
                name="ccdram", bufs=1, space="DRAM"
            ) as ccd, tc.tile_pool(name="tp_ps", bufs=3, space="PSUM") as tp_ps, tc.tile_pool(
                name="pr_ps", bufs=3, space="PSUM"
            ) as pr_ps:
                # local (this half's) projected k^T and v, pre-gather
                kloc = p1.tile([P, D // P, SQ], f16, name="kloc")
                vloc = p1.tile([P, SQ // P, NH, HK], f16, name="vloc")
                in_k = ccd.tile([P, D // P, SQ], f16, name="in_k")
                out_k = ccd.tile([2, P, D // P, SQ], f16, name="out_k")
                in_v = ccd.tile([P, SQ // P, NH, HK], f16, name="in_v")
                out_v = ccd.tile([2, P, SQ // P, NH, HK], f16, name="out_v")
                groups = [[0, 1], [2, 3], [4, 5], [6, 7]]
                for t, (xdram, wdram) in enumerate(
                    ((xk, wk), (xv, wv), (xq, wq))
                ):
                    w_sb = p1.tile([P, D // P, D], f16, name=f"w{t}", tag="w", bufs=2)
                    nc.sync.dma_start(
                        w_sb[:], wdram.rearrange("(dc p) n -> p dc n", p=P)
                    )
                    for st in range(SQ // QT):
                        # stage 4 row-blocks of X
                        stg = []
                        for sb in range(4):
                            xst = p1.tile([P, D], f16, name="xst", tag="xst", bufs=5)
                            nc.sync.dma_start(
                                xst[:], xdram[ds(st * QT + sb * P, P), :]
                            )
                            stg.append(xst)
                        # transpose to X^T tile [128 d, dchunk, 512 s]
                        xt = p1.tile([P, D // P, QT], f16, name="xt", tag="xt", bufs=2)
                        for dc in range(D // P):
                            ptp = tp_ps.tile([P, QT], f16, name="ptp")
                            for sb in range(4):
                                nc.tensor.transpose(
                                    ptp[:, ds(sb * P, P)],
                                    stg[sb][:, ds(dc * P, P)],
                                    idn[:],
                                )
                            nc.scalar.copy(xt[:, dc, :], ptp[:])
                        if t != 1:
                            # q^T / k^T orientation: psum [128 dk, 512 s]
                            for dkb in range(D // P):
                                ppr = pr_ps.tile([P, QT], f32, name="ppr")
                                for dc in range(D // P):
                                    nc.tensor.matmul(
                                        ppr[:],
                                        w_sb[:, dc, ds(dkb * P, P)],
                                        xt[:, dc, :],
                                        start=(dc == 0),
                                        stop=(dc == D // P - 1),
                                    )
                                if t == 2:
                                    nc.vector.tensor_copy(
                                        qpad[0:64, 2 * dkb, ds(st * QT, QT)],
                                        ppr[0:64, :],
                                    )
                                    nc.vector.tensor_copy(
                                        qpad[64:128, 2 * dkb + 1, ds(st * QT, QT)],
                                        ppr[64:128, :],
                                    )
                                else:
                                    nc.vector.tensor_copy(
                                        kloc[:, dkb, ds(st * QT, QT)], ppr[:]
                                    )
                        else:
                            # v orientation: psum [128 s, 512 dk] per half
                            for sb in range(4):
                                ci = st * 4 + sb
                                for vh in range(2):
                                    ppr = pr_ps.tile([P, QT], f32, name="ppr")
                                    for dc in range(D // P):
                                        nc.tensor.matmul(
                                            ppr[:],
                                            xt[:, dc, ds(sb * P, P)],
                                            w_sb[:, dc, ds(vh * QT, QT)],
                                            start=(dc == 0),
                                            stop=(dc == D // P - 1),
                                        )
                                    nc.vector.tensor_copy(
                                        vloc[:, ci, ds(vh * 8, 8), :],
                                        ppr[:].rearrange("p (h k) -> p h k", k=HK),
                                    )
                    if t == 0:
                        # launch k AllGather as soon as k is projected
                        nc.sync.dma_start(in_k[:], kloc[:])
                        nc.gpsimd.collective_compute(
                            "AllGather",
                            mybir.AluOpType.bypass,
                            replica_groups=groups,
                            ins=[in_k.opt()],
                            outs=[out_k.opt()],
                        )
                        for r in range(2):
                            nc.sync.dma_start(kT[:, :, ds(r * SQ, SQ)], out_k[r])
                    elif t == 1:
                        nc.sync.dma_start(in_v[:], vloc[:])
                        nc.gpsimd.collective_compute(
                            "AllGather",
                            mybir.AluOpType.bypass,
                            replica_groups=groups,
                            ins=[in_v.opt()],
                            outs=[out_v.opt()],
                        )
                        for r in range(2):
                            nc.sync.dma_start(
                                vaug[:, ds(r * (SQ // P), SQ // P), :, 0:HK], out_v[r]
                            )

            # ---------------- phase 2: attention + output projection ----------
            with tc.tile_pool(name="p2sb", bufs=1) as p2, tc.tile_pool(
                name="et_pool", bufs=3
            ) as etp, tc.tile_pool(name="bc_pool", bufs=2) as bcp, tc.tile_pool(
                name="cu_pool", bufs=1
            ) as cup, tc.tile_pool(name="ct_pool", bufs=2) as ctp, tc.tile_pool(
                name="ob_pool", bufs=2
            ) as obp, tc.tile_pool(
                name="sc_ps", bufs=2, space="PSUM"
            ) as sc_ps, tc.tile_pool(
                name="ct_ps", bufs=1, space="PSUM"
            ) as ct_ps, tc.tile_pool(
                name="op_ps", bufs=1, space="PSUM"
            ) as op_ps, tc.tile_pool(
                name="bc_ps", bufs=2, space="PSUM"
            ) as bc_ps:
                wot_sb = p2.tile([P, D // P, D], f16, name="wot_sb")
                nc.sync.dma_start(
                    wot_sb[:], wot.rearrange("(c p) n -> p c n", p=P)
                )
                bo_sb = p2.tile([1, D], f16, name="bo_sb")
                nc.sync.dma_start(bo_sb[:], bo[:, :])
                for qt in range(NQT):
                    ctxU = cup.tile([P, D // P, QT], f32, name="ctxU")
                    ctxT = ctp.tile([P, D // P, QT], f16, name="ctxT")
                    for h in range(NH):
                        p = h // 2
                        pct = ct_ps.tile([HK + 1, QT], f32, name="pct")
                        for g in range(NKC // 2):
                            psc = sc_ps.tile([P, 2, QT], f32, name="psc")
                            for j in range(2):
                                ch = 2 * g + j
                                nc.tensor.matmul(
                                    psc[:, j, :],
                                    kT[:, p, ds(ch * P, P)],
                                    qpad[:, h, ds(qt * QT, QT)],
                                    start=True,
                                    stop=True,
                                )
                            ete = etp.tile([P, 2, QT], f16, name="ete")
                            nc.scalar.activation(ete[:], psc[:], EXP, scale=SCALE)
                            for j in range(2):
                                ch = 2 * g + j
                                nc.tensor.matmul(
                                    pct[:],
                                    vaug[:, ch, h, :],
                                    ete[:, j, :],
                                    start=(ch == 0),
                                    stop=(ch == NKC - 1),
                                )
                        sl = slice(64 * (h % 2), 64 * (h % 2) + 64)
                        nc.vector.tensor_copy(ctxU[sl, h // 2, :], pct[0:64, :])
                        dln = bcp.tile([1, QT], f32, name="dln", tag="dln", bufs=3)
                        nc.scalar.activation(dln[:], pct[64:65, :], LN)
                        drec = bcp.tile([1, QT], f16, name="drec", tag="drec", bufs=3)
                        nc.scalar.activation(drec[:], dln[:], EXP, scale=-1.0)
                        bch = bc_ps.tile([P, QT], f32, name="bch")
                        nc.tensor.matmul(bch[:], ones1[:], drec[:], start=True, stop=True)
                        nc.vector.tensor_mul(
                            out=ctxT[sl, h // 2, :],
                            in0=ctxU[sl, h // 2, :],
                            in1=bch[sl, :],
                        )
                    for sb in range(QT // P):
                        ob = obp.tile([P, 2, QT], f16, name="ob")
                        for nh in range(2):
                            pop = op_ps.tile([P, QT], f32, name="pop")
                            # rank-1 bias matmul seeds the accumulation
                            nc.tensor.matmul(
                                pop[:],
                                ones1[:],
                                bo_sb[0:1, ds(nh * QT, QT)],
                                start=True,
                                stop=False,
                            )
                            for cc in range(D // P):
                                nc.tensor.matmul(
                                    pop[:],
                                    ctxT[:, cc, ds(sb * P, P)],
                                    wot_sb[:, cc, ds(nh * QT, QT)],
                                    start=False,
                                    stop=(cc == D // P - 1),
                                )
                            nc.vector.tensor_copy(ob[:, nh, :], pop[:])
                        nc.sync.dma_start(
                            out[ds(qt * QT + sb * P, P), :],
                            ob[:].rearrange("p a b -> p (a b)"),
                        )
    nc.compile()
    return nc


_NC_CACHE = {}


def _weights_key(Wq, Wk, Wv, Wo, bo):
    import hashlib

    h = hashlib.sha1()
    for a in (Wq, Wk, Wv, Wo, bo):
        h.update(np.ascontiguousarray(a).tobytes())
    return h.hexdigest()


def _get_nc(Wq, Wk, Wv, Wo, bo):
    key = _weights_key(Wq, Wk, Wv, Wo, bo)
    if key not in _NC_CACHE:
        WQ = np.ascontiguousarray(
            np.asarray(Wq, np.float32).transpose(1, 0, 2).reshape(D, NH * HK)
        ).astype(np.float16)
        WK = np.ascontiguousarray(
            np.asarray(Wk, np.float32).transpose(1, 0, 2).reshape(D, NH * HK)
        ).astype(np.float16)
        WV = np.ascontiguousarray(
            np.asarray(Wv, np.float32).transpose(1, 0, 2).reshape(D, NH * HK)
        ).astype(np.float16)
        WOT = np.ascontiguousarray(np.asarray(Wo, np.float32).T).astype(np.float16)
        BO = np.asarray(bo, np.float32).reshape(1, D).astype(np.float16)
        _NC_CACHE.clear()  # one compiled program at a time
        _NC_CACHE[key] = build_nc(WQ, WK, WV, WOT, BO)
    return _NC_CACHE[key]


def make_in_maps(query, key_in, value, Wq, Wk, Wv, Wo, bo):
    query = np.asarray(query, dtype=np.float16)
    key_in = np.asarray(key_in, dtype=np.float16)
    value = np.asarray(value, dtype=np.float16)
    in_maps = []
    for c in range(8):
        b = c // 2
        half = c % 2
        sl = slice(half * SQ, (half + 1) * SQ)
        in_maps.append(
            {
                "xq": np.ascontiguousarray(query[b, sl]),
                "xk": np.ascontiguousarray(key_in[b, sl]),
                "xv": np.ascontiguousarray(value[b, sl]),
            }
        )
    return in_maps


_EXEC_CACHE = {}


def _get_exec(nc):
    """Build (once per nc) the jitted 8-core shard_map executable around
    _bass_exec_p, mirroring run_bass_via_pjrt. A single executable per
    process keeps the collective mesh in sync across repeated calls."""
    key = id(nc)
    if key in _EXEC_CACHE:
        return _EXEC_CACHE[key]
    import jax
    from jax.sharding import Mesh, PartitionSpec
    from jax.experimental.shard_map import shard_map
    from concourse import bass2jax

    bass2jax.install_neuronx_cc_hook()
    n_cores = 8
    partition_name = nc.partition_id_tensor.name if nc.partition_id_tensor else None
    in_names, out_names, out_avals, zero_outs = [], [], [], []
    for alloc in nc.m.functions[0].allocations:
        if not isinstance(alloc, mybir.MemoryLocationSet):
            continue
        name = alloc.memorylocations[0].name
        if alloc.kind == "ExternalInput":
            if name != partition_name:
                in_names.append(name)
        elif alloc.kind == "ExternalOutput":
            out_names.append(name)
            shape = tuple(alloc.tensor_shape)
            dtype = mybir.dt.np(alloc.dtype)
            out_avals.append(jax.core.ShapedArray(shape, dtype))
            zero_outs.append(np.zeros(shape, dtype))
    n_params = len(in_names)
    n_outs = len(out_avals)
    in_names_all = in_names + out_names + (
        [partition_name] if partition_name else []
    )
    donate = tuple(range(n_params, n_params + n_outs))

    def _body(*args):
        operands = list(args)
        if partition_name is not None:
            operands.append(bass2jax.partition_id_tensor())
        outs = bass2jax._bass_exec_p.bind(
            *operands,
            out_avals=tuple(out_avals),
            in_names=tuple(in_names_all),
            out_names=tuple(out_names),
            lowering_input_output_aliases=(),
            sim_require_finite=True,
            sim_require_nnan=True,
            nc=nc,
        )
        return tuple(outs)

    devices = jax.devices()[:n_cores]
    mesh = Mesh(np.asarray(devices), ("core",))
    in_specs = (PartitionSpec("core"),) * (n_params + n_outs)
    out_specs = (PartitionSpec("core"),) * len(out_names)
    sharded = jax.jit(
        shard_map(
            _body, mesh=mesh, in_specs=in_specs, out_specs=out_specs,
            check_rep=False,
        ),
        donate_argnums=donate,
        keep_unused=True,
    )
    _EXEC_CACHE.clear()
    _EXEC_CACHE[key] = (sharded, in_names, out_names, out_avals, zero_outs)
    return _EXEC_CACHE[key]


def kernel(query, key_in, value, Wq, Wk, Wv, Wo, bo):
    nc = _get_nc(Wq, Wk, Wv, Wo, bo)
    sharded, in_names, out_names, out_avals, zero_outs = _get_exec(nc)
    in_maps = make_in_maps(query, key_in, value, Wq, Wk, Wv, Wo, bo)
    n_cores = 8
    concat_in = [
        np.concatenate([in_maps[c][nm] for c in range(n_cores)], axis=0)
        for nm in in_names
    ]
    concat_zeros = [
        np.zeros((n_cores * z.shape[0], *z.shape[1:]), z.dtype) for z in zero_outs
    ]
    out_arrs = sharded(*concat_in, *concat_zeros)
    res = np.asarray(out_arrs[out_names.index("out")]).reshape(n_cores, SQ, D)
    B = np.asarray(query).shape[0]
    out = np.empty((B, S, D), dtype=np.float32)
    for c in range(n_cores):
        b = c // 2
        half = c % 2
        out[b, half * SQ : (half + 1) * SQ] = res[c]
    return out


if __name__ == "__main__":
    rng = np.random.default_rng(0)
    q = rng.standard_normal((4, S, D), dtype=np.float32)
    k = rng.standard_normal((4, S, D), dtype=np.float32)
    v = rng.standard_normal((4, S, D), dtype=np.float32)
    sd = 1.0 / np.sqrt(D)
    Wq = rng.standard_normal((16, D, HK), dtype=np.float32) * sd
    Wk = rng.standard_normal((16, D, HK), dtype=np.float32) * sd
    Wv = rng.standard_normal((16, D, HK), dtype=np.float32) * sd
    Wo = rng.standard_normal((D, D), dtype=np.float32) * sd
    bo = rng.standard_normal((D,), dtype=np.float32) * 0.01
    o = kernel(q, k, v, Wq, Wk, Wv, Wo, bo)
    print("out", o.shape, o.dtype, np.abs(o).max())
